# revision 1
# baseline (speedup 1.0000x reference)
"""Multi-head attention (B=2, T=2048, H=1024, 16 heads) on 8 trn2 cores.

Sharding: data-parallel over batch (2) x tensor-parallel over head groups
(4 heads/core).  Each core computes the qkv projection for its 4 heads,
attention, and a partial out-projection; the host sums the 4 partials per
batch and adds b_out.

Device layout (per core):
  xT   [1024, 2048]  (x transposed on-device, built in two T-halves)
  qkvT [768, 2048]   = W_sliceT . xT  (6 M-tiles: q01 k01 v01 q23 k23 v23,
                       emitted in dependency order so attention on heads 0-1
                       overlaps the projection of heads 2-3)
  per head: scoresT [Tk, Tq] = kT.T@qT (K=64), mask+scale+exp as one ACT op
  with per-partition mask bias on [128,1024] tiles, AV matmul with
  ones-augmented V giving unnormalized outT + softmax denominator in PSUM,
  reciprocal + PE-broadcast + DVE multiply for normalization,
  out-projection (K=128 over head pairs) -> partial [2048, 1024].
Matmuls run in float32r (full-rate fp32 path, producers write rounded).
One shared PSUM pool with per-tag slots lets projection and attention
pipelines coexist without bank conflicts (8 banks exactly).
"""

import sys

sys.path.insert(0, "/opt/trn_rl_repo")

import numpy as np

B, T, H = 2, 2048, 1024
NH, DK = 16, 64
HPC = 4           # heads per core
F = 3 * HPC * DK  # 768 qkv features per core
NCORES = 8

_CACHE = {}


def _build(reps=1):
    import concourse.bacc as bacc
    import concourse.mybir as mybir
    import concourse.tile as tile
    from concourse.masks import make_identity

    f32 = mybir.dt.float32
    f32r = mybir.dt.float32r
    AF = mybir.ActivationFunctionType
    ALU = mybir.AluOpType

    nc = bacc.Bacc("TRN2", target_bir_lowering=False, debug=False)

    x_d = nc.dram_tensor("x", [T, H], f32, kind="ExternalInput")
    wqkv_d = nc.dram_tensor("w_qkv", [H, F], f32r, kind="ExternalInput")
    bqkvt_d = nc.dram_tensor("b_qkvt", [128, 6], f32, kind="ExternalInput")
    wout_d = nc.dram_tensor("w_out", [HPC * DK, H], f32r, kind="ExternalInput")
    maskb_d = nc.dram_tensor("maskbias", [128, 16], f32, kind="ExternalInput")
    out_d = nc.dram_tensor("out_partial", [T, H], f32, kind="ExternalOutput")

    NT = T // 128   # 16 token tiles
    KT = H // 128   # 8 contraction tiles for projection
    TH = T // 2     # T-half for xT staging

    with tile.TileContext(nc) as tc:
        with (
            tc.tile_pool(name="persist", bufs=1) as pp,
            tc.tile_pool(name="small", bufs=1) as sp,
            tc.tile_pool(name="xload", bufs=2) as xp,
            tc.tile_pool(name="xT_pool", bufs=1) as xtp,
            tc.tile_pool(name="w_pool", bufs=1) as wp,
            tc.tile_pool(name="expp", bufs=4) as ep,
            tc.tile_pool(name="recipp", bufs=4) as rp,
            tc.tile_pool(name="ostage", bufs=4) as osp,
            tc.tile_pool(name="psum", bufs=1, space="PSUM") as psp,
        ):
            for rep in range(reps):
                ident = sp.tile([128, 128], f32, tag="ident", name="ident")
                make_identity(nc, ident)
                ones_f = sp.tile([128, 64], f32, tag="ones_f", name="ones_f")
                nc.vector.memset(ones_f, 1.0)
                ones = sp.tile([1, 64], f32r, tag="ones", name="ones")
                nc.scalar.copy(ones, ones_f[0:1, :])
                bqkvt = sp.tile([128, 6], f32, tag="bqkvt", name="bqkvt")
                nc.sync.dma_start(out=bqkvt, in_=bqkvt_d[:, :])
                maskb = sp.tile([128, 16], f32, tag="maskb", name="maskb")
                nc.sync.dma_start(out=maskb, in_=maskb_d[:, :])
                wout = sp.tile([128, 2 * H], f32r, tag="wout", name="wout")
                for p in range(2):
                    nc.gpsimd.dma_start(
                        out=wout[:, p * H : (p + 1) * H],
                        in_=wout_d[p * 128 : (p + 1) * 128, :],
                    )
                wq = wp.tile([128, KT * F], f32r, tag="wq", name="wq")
                for kt in range(KT):
                    nc.gpsimd.dma_start(
                        out=wq[:, kt * F : (kt + 1) * F],
                        in_=wqkv_d[kt * 128 : (kt + 1) * 128, :],
                    )

                # qkvT M-tiles in emission order: q01 k01 v01 q23 k23 v23
                # (host arranges w_qkv columns to match).
                qkvT = [
                    pp.tile([128, T], f32r, tag=f"qkvT{m}", name=f"qkvT{m}")
                    for m in range(6)
                ]
                Q = {0: qkvT[0], 1: qkvT[3]}   # head-pair -> qT tile
                Kt = {0: qkvT[1], 1: qkvT[4]}  # head-pair -> kT tile
                V = {0: qkvT[2], 1: qkvT[5]}   # head-pair -> vT tile

                vp = [
                    pp.tile([128, NT * 65], f32r, tag=f"vp{h}", name=f"vp{h}")
                    for h in range(HPC)
                ]
                attn = [
                    pp.tile([128, T], f32r, tag=f"attnp{p}", name=f"attnp{p}")
                    for p in range(2)
                ]

                def build_xT_half(half):
                    xT = xtp.tile([128, KT * TH], f32r, tag="xT", name="xT")
                    for lt in range(8):
                        tt = half * 8 + lt
                        xtile = xp.tile([128, H], f32, tag="xtile", name="xtile")
                        for ch in range(2):
                            nc.sync.dma_start(
                                out=xtile[:, ch * 512 : (ch + 1) * 512],
                                in_=x_d[
                                    tt * 128 : (tt + 1) * 128,
                                    ch * 512 : (ch + 1) * 512,
                                ],
                            )
                        for kt in range(KT):
                            pt = psp.tile(
                                [128, 128], f32, tag="ss", bufs=2, name="pt"
                            )
                            nc.tensor.transpose(
                                pt, xtile[:, kt * 128 : (kt + 1) * 128], ident
                            )
                            nc.vector.tensor_copy(
                                xT[:, kt * TH + lt * 128 : kt * TH + (lt + 1) * 128],
                                pt,
                            )
                    return xT

                def project(xT, half, mts):
                    for mt in mts:
                        for lnb in range(2):
                            ps = psp.tile(
                                [128, 512], f32, tag="ps", bufs=2, name="ps"
                            )
                            for kt in range(KT):
                                nc.tensor.matmul(
                                    ps,
                                    wq[
                                        :,
                                        kt * F + mt * 128 : kt * F + (mt + 1) * 128,
                                    ],
                                    xT[
                                        :,
                                        kt * TH + lnb * 512 : kt * TH
                                        + lnb * 512
                                        + 512,
                                    ],
                                    start=(kt == 0),
                                    stop=(kt == KT - 1),
                                )
                            nc.scalar.activation(
                                qkvT[mt][
                                    :,
                                    half * TH + lnb * 512 : half * TH
                                    + lnb * 512
                                    + 512,
                                ],
                                ps,
                                AF.Identity,
                                bias=bqkvt[:, mt : mt + 1],
                                scale=1.0,
                            )

                def build_vp(hp):
                    # v' = [v | 1] per head of pair hp, from V[hp]
                    for lh in range(2):
                        h = hp * 2 + lh
                        r0 = lh * 64
                        vpv = vp[h].rearrange("p (t c) -> p t c", c=65)
                        nc.scalar.copy(vpv[:, :, 64], ones_f[:, 0:16])
                        for kt in range(NT):
                            pv = psp.tile(
                                [128, 64], f32, tag="ps", bufs=2, name="pv"
                            )
                            nc.tensor.transpose(
                                pv,
                                V[hp][r0 : r0 + 64, kt * 128 : (kt + 1) * 128]
                                .bitcast(f32),
                                ident[r0 : r0 + 64, r0 : r0 + 64],
                            )
                            nc.vector.tensor_copy(
                                vp[h][:, kt * 65 : kt * 65 + 64], pv
                            )

                def attention_pair(hp):
                    # both heads of the pair run together: their K=64 score
                    # matmuls occupy disjoint PE row groups and write the two
                    # banks of one [128,1024] PSUM tile, so a single exp
                    # activation (mask bias depends only on kt) serves both.
                    for nb in range(4):  # 512-wide query blocks
                        accs = [
                            psp.tile(
                                [65, 512], f32, tag="acc", bufs=2,
                                name="acc",
                            )
                            for lh in range(2)
                        ]
                        for kt in range(NT):
                            ss = psp.tile(
                                [128, 1024], f32, tag="ss", bufs=2, name="ss"
                            )
                            for lh in range(2):
                                r0 = lh * 64
                                nc.tensor.matmul(
                                    ss[:, lh * 512 : (lh + 1) * 512],
                                    Kt[hp][
                                        r0 : r0 + 64,
                                        kt * 128 : (kt + 1) * 128,
                                    ],
                                    Q[hp][
                                        r0 : r0 + 64,
                                        nb * 512 : nb * 512 + 512,
                                    ],
                                    start=True,
                                    stop=True,
                                )
                            ex = ep.tile([128, 1024], f32r, tag="ex", name="ex")
                            nc.scalar.activation(
                                ex,
                                ss,
                                AF.Exp,
                                bias=maskb[:, kt : kt + 1],
                                scale=0.125,
                            )
                            for lh in range(2):
                                h = hp * 2 + lh
                                nc.tensor.matmul(
                                    accs[lh],
                                    vp[h][:, kt * 65 : kt * 65 + 65],
                                    ex[:, lh * 512 : (lh + 1) * 512],
                                    start=(kt == 0),
                                    stop=(kt == NT - 1),
                                )
                        for lh in range(2):
                            r0 = lh * 64
                            acc = accs[lh]
                            rec = rp.tile([1, 512], f32r, tag="rec", name="rec")
                            with nc.allow_low_precision(
                                reason="f32r rounding for PE broadcast"
                            ):
                                nc.vector.reciprocal(rec, acc[64:65, :])
                            pb = psp.tile(
                                [64, 512], f32, tag="ps", bufs=2, name="pb"
                            )
                            nc.tensor.matmul(
                                pb, ones, rec, start=True, stop=True
                            )
                            recb = rp.tile(
                                [64, 512], f32, tag="recb", name="recb"
                            )
                            nc.vector.tensor_copy(recb, pb)
                            nc.vector.tensor_tensor(
                                out=attn[hp][
                                    r0 : r0 + 64,
                                    nb * 512 : nb * 512 + 512,
                                ],
                                in0=acc[0:64, :],
                                in1=recb,
                                op=ALU.mult,
                            )


                # ---- schedule ----
                xT0 = build_xT_half(0)
                project(xT0, 0, range(6))
                xT1 = build_xT_half(1)
                project(xT1, 1, [0, 1, 2])   # q01 k01 v01 complete here
                build_vp(0)
                project(xT1, 1, [3, 4, 5])   # overlaps attention h0-h1
                attention_pair(0)
                build_vp(1)
                attention_pair(1)

                # ---- out projection (K=128 over head pairs) ----
                for mt in range(NT):
                    for ob in range(2):
                        po = psp.tile([128, 512], f32, tag="ss", bufs=2, name="po")
                        for p in range(2):
                            nc.tensor.matmul(
                                po,
                                attn[p][:, mt * 128 : (mt + 1) * 128],
                                wout[:, p * H + ob * 512 : p * H + ob * 512 + 512],
                                start=(p == 0),
                                stop=(p == 1),
                            )
                        ot = osp.tile([128, 512], f32, tag="ot", name="ot")
                        nc.any.tensor_copy(ot, po)
                        nc.sync.dma_start(
                            out=out_d[
                                mt * 128 : (mt + 1) * 128,
                                ob * 512 : ob * 512 + 512,
                            ],
                            in_=ot,
                        )

    nc.compile()
    return nc


def _get_nc(reps=1):
    key = f"nc{reps}"
    if key not in _CACHE:
        _CACHE[key] = _build(reps)
    return _CACHE[key]


def _prep_in_maps(x, mask, W_qkv, b_qkv, W_out):
    in_maps = []
    for c in range(NCORES):
        b = c // 4
        h0 = (c % 4) * HPC
        # column order q01 k01 v01 q23 k23 v23 (matching device M-tiles)
        blocks = []
        for pair in range(2):
            for sec in range(3):  # q, k, v
                lo = sec * H + (h0 + pair * 2) * DK
                blocks.append(np.arange(lo, lo + 2 * DK))
        cols = np.concatenate(blocks)
        w_slice = np.ascontiguousarray(W_qkv[:, cols])
        b_slice = np.ascontiguousarray(b_qkv[cols])
        bqkvt = np.ascontiguousarray(b_slice.reshape(6, 128).T)
        w_out_slice = np.ascontiguousarray(W_out[h0 * DK : (h0 + HPC) * DK, :])
        mb = (mask[b, 0, 0, :].astype(np.float32) - 1.0) * 1e9
        mb = np.ascontiguousarray(mb.reshape(16, 128).T)
        in_maps.append(
            {
                "x": np.ascontiguousarray(x[b]),
                "w_qkv": w_slice,
                "b_qkvt": bqkvt,
                "w_out": w_out_slice,
                "maskbias": mb,
            }
        )
    return in_maps


def _combine(partials, b_out):
    out = np.empty((B, T, H), dtype=np.float32)
    for b in range(B):
        acc = partials[4 * b].astype(np.float32)
        for i in range(1, 4):
            acc = acc + partials[4 * b + i]
        out[b] = acc + b_out[None, :]
    return out


def kernel(x, mask, W_qkv, b_qkv, W_out, b_out):
    x = np.asarray(x, dtype=np.float32)
    mask = np.asarray(mask)
    W_qkv = np.asarray(W_qkv, dtype=np.float32)
    b_qkv = np.asarray(b_qkv, dtype=np.float32)
    W_out = np.asarray(W_out, dtype=np.float32)
    b_out = np.asarray(b_out, dtype=np.float32)

    nc = _get_nc()
    in_maps = _prep_in_maps(x, mask, W_qkv, b_qkv, W_out)

    from concourse.bass_utils import run_bass_kernel_spmd

    res = run_bass_kernel_spmd(nc, in_maps, list(range(NCORES)))
    partials = [res.results[c]["out_partial"] for c in range(NCORES)]
    return _combine(partials, b_out)



# revision 12
# speedup vs baseline: 2.0909x; 2.0909x over previous
"""Multi-head attention (B=2, T=2048, H=1024, 16 heads) on 8 trn2 cores.

Sharding: data-parallel over batch (2) x tensor-parallel over head groups
(4 heads/core).  Each core computes qkv projection for its 4 heads,
attention, and a partial out-projection; the host sums 4 partials per
batch and adds b_out.

Key structural choices vs the straightforward version:
  * Host-side transpose: x arrives as xT [H, T] so no on-device PE
    transposes / DVE copies are needed to stage the projection rhs.
  * Mask-driven key compaction: the boolean mask kills ~half the keys
    exactly (exp(-1e9) == 0 in f32), so the host gathers only valid key
    rows into x_kv (padded to a multiple of 128).  Scores, exp and AV
    run on ~half the key dim; padded tail keys get a -1e9 bias so they
    contribute exactly 0, and their vp ones-column entry is 0.
  * V is produced directly in key-major orientation (lhsT = xT_kv
    chunk, rhs = W_v slice), so no V transpose either.
  * bf16 operands everywhere on the matmul paths (f32 PSUM accum),
    halving DMA and SBUF; maskbias/denominators stay f32.
  * Normalization: denominator row from the ones-augmented AV matmul,
    reciprocal_approx_fast on DVE, gpsimd partition_broadcast, DVE
    multiply.  No 3us single-lane reciprocal, no PE broadcast matmul.
  * Out-projection interleaved per 512-query block; a short burst of
    dummy warm-up matmuls at t=0 keeps the PE HAM clock from starting
    cold during the input DMA.
"""

import sys

sys.path.insert(0, "/opt/trn_rl_repo")

import numpy as np
from ml_dtypes import bfloat16

B, T, H = 2, 2048, 1024
NH, DK = 16, 64
HPC = 4           # heads per core
NCORES = 8
NB = T // 512     # query blocks
KT = H // 128     # contraction tiles for projections

_CACHE = {}


def _build(nv_pad):
    import concourse.bacc as bacc
    import concourse.mybir as mybir
    import concourse.tile as tile

    f32 = mybir.dt.float32
    bf16 = mybir.dt.bfloat16
    AF = mybir.ActivationFunctionType
    ALU = mybir.AluOpType

    NKT = nv_pad // 128   # key tiles

    nc = bacc.Bacc("TRN2", target_bir_lowering=False, debug=False)

    xT_d = nc.dram_tensor("xT", [H, T], bf16, kind="ExternalInput")
    xkvT_d = nc.dram_tensor("xkvT", [H, nv_pad], bf16, kind="ExternalInput")
    wq_d = nc.dram_tensor("w_qk", [H, 512], bf16, kind="ExternalInput")
    wv_d = nc.dram_tensor("w_v", [H, 256], bf16, kind="ExternalInput")
    wout_d = nc.dram_tensor("w_out", [2 * 128, H], bf16, kind="ExternalInput")
    maskb_d = nc.dram_tensor("maskbias", [128, NKT], f32, kind="ExternalInput")
    vones_d = nc.dram_tensor("validones", [128, NKT], f32, kind="ExternalInput")
    bqk_d = nc.dram_tensor("b_qk", [128, 4], f32, kind="ExternalInput")
    bvb_d = nc.dram_tensor("b_vb", [128, 256], f32, kind="ExternalInput")
    out_d = nc.dram_tensor("out_partial", [T, H], f32, kind="ExternalOutput")

    kv_blocks = [(o, min(512, nv_pad - o)) for o in range(0, nv_pad, 512)]

    with tile.TileContext(nc) as tc:
        with (
            tc.tile_pool(name="persist", bufs=1) as pp,
            tc.tile_pool(name="expp", bufs=4) as ep,
            tc.tile_pool(name="recp", bufs=4) as rp,
            tc.tile_pool(name="ostage", bufs=4) as osp,
            tc.tile_pool(name="psum", bufs=1, space="PSUM") as psp,
        ):
            # ---- persistent SBUF tiles ----
            scratch = pp.tile([128, 256], bf16, tag="scratch", name="scratch")
            nc.vector.memset(scratch, 0.125)
            wq = pp.tile([128, KT * 512], bf16, tag="wq", name="wq")
            wv = pp.tile([128, KT * 256], bf16, tag="wv", name="wv")
            wout = pp.tile([128, 2 * H], bf16, tag="wout", name="wout")
            maskb = pp.tile([128, NKT], f32, tag="maskb", name="maskb")
            vones = pp.tile([128, NKT], f32, tag="vones", name="vones")
            bqk = pp.tile([128, 4], f32, tag="bqk", name="bqk")
            bvb = pp.tile([128, 256], f32, tag="bvb", name="bvb")
            xkvT = pp.tile([128, KT * nv_pad], bf16, tag="xkvT", name="xkvT")
            xT = pp.tile([128, KT * T], bf16, tag="xT", name="xT")
            qT = [pp.tile([128, T], bf16, tag=f"qT{p}", name=f"qT{p}")
                  for p in range(2)]
            kT = [pp.tile([128, nv_pad], bf16, tag=f"kT{p}", name=f"kT{p}")
                  for p in range(2)]
            vp = [pp.tile([128, NKT * 65], bf16, tag=f"vp{h}", name=f"vp{h}")
                  for h in range(HPC)]
            attn = [pp.tile([128, T], bf16, tag=f"attn{p}", name=f"attn{p}")
                    for p in range(2)]
            # denominator staging at partitions {0,32} (SBUF AP start rule);
            # persistent + memset once so rows 1..31 are defined for the
            # batched reciprocal
            dd = pp.tile([33, 512], f32, tag="dd", name="dd")
            nc.vector.memset(dd, 1.0)
            rr = pp.tile([33, 512], f32, tag="rr", name="rr")

            # ---- PE warm-up: keep HAM busy while input DMAs run ----
            for i in range(40):
                wps = psp.tile([128, 1024], f32, tag="ss", bufs=2, name="wps")
                nc.tensor.matmul(
                    wps[:, 0:128], scratch[:, 0:128], scratch[:, 128:256],
                    start=True, stop=True,
                )

            # ---- input DMAs ----
            nc.gpsimd.dma_start(out=maskb, in_=maskb_d[:, :])
            nc.gpsimd.dma_start(out=vones, in_=vones_d[:, :])
            nc.gpsimd.dma_start(out=bqk, in_=bqk_d[:, :])
            nc.gpsimd.dma_start(out=bvb, in_=bvb_d[:, :])
            for kt in range(KT):
                nc.gpsimd.dma_start(
                    out=wq[:, kt * 512:(kt + 1) * 512],
                    in_=wq_d[kt * 128:(kt + 1) * 128, :],
                )
                nc.gpsimd.dma_start(
                    out=wv[:, kt * 256:(kt + 1) * 256],
                    in_=wv_d[kt * 128:(kt + 1) * 128, :],
                )
            for p in range(2):
                nc.gpsimd.dma_start(
                    out=wout[:, p * H:(p + 1) * H],
                    in_=wout_d[p * 128:(p + 1) * 128, :],
                )
            for kt in range(KT):
                nc.sync.dma_start(
                    out=xkvT[:, kt * nv_pad:(kt + 1) * nv_pad],
                    in_=xkvT_d[kt * 128:(kt + 1) * 128, :],
                )
            for kt in range(KT):
                nc.sync.dma_start(
                    out=xT[:, kt * T:(kt + 1) * T],
                    in_=xT_d[kt * 128:(kt + 1) * 128, :],
                )

            # vp ones columns (0 for padded key rows)
            for h in range(HPC):
                vpv = vp[h].rearrange("p (t c) -> p t c", c=65)
                nc.gpsimd.tensor_copy(vpv[:, :, 64], vones)

            # ---- projections ----
            def proj_k(pair):
                # kT[pair][dk(128), keys] = W_k.T @ x_kv ; M-tile col offset
                mt = 2 + pair
                for (o, w) in kv_blocks:
                    ps = psp.tile([128, 512], f32, tag="ps", bufs=2, name="ps")
                    for kt in range(KT):
                        nc.tensor.matmul(
                            ps[:, 0:w],
                            wq[:, kt * 512 + mt * 128: kt * 512 + (mt + 1) * 128],
                            xkvT[:, kt * nv_pad + o: kt * nv_pad + o + w],
                            start=(kt == 0), stop=(kt == KT - 1),
                        )
                    nc.vector.tensor_scalar_add(
                        kT[pair][:, o:o + w], ps[:, 0:w], bqk[:, mt:mt + 1],
                    )

            def proj_q(pair, nb):
                mt = pair
                ps = psp.tile([128, 512], f32, tag="ps", bufs=2, name="ps")
                for kt in range(KT):
                    nc.tensor.matmul(
                        ps,
                        wq[:, kt * 512 + mt * 128: kt * 512 + (mt + 1) * 128],
                        xT[:, kt * T + nb * 512: kt * T + nb * 512 + 512],
                        start=(kt == 0), stop=(kt == KT - 1),
                    )
                nc.vector.tensor_scalar_add(
                    qT[pair][:, nb * 512:(nb + 1) * 512], ps, bqk[:, mt:mt + 1],
                )

            def proj_v():
                # key-major: v[key, dk4] = x_kv @ W_v, one keytile at a time
                for kb in range(NKT):
                    ps = psp.tile([128, 512], f32, tag="ps", bufs=2, name="pv")
                    for kt in range(KT):
                        nc.tensor.matmul(
                            ps[:, 0:256],
                            xkvT[:, kt * nv_pad + kb * 128:
                                 kt * nv_pad + (kb + 1) * 128],
                            wv[:, kt * 256:(kt + 1) * 256],
                            start=(kt == 0), stop=(kt == KT - 1),
                        )
                    for h in range(HPC):
                        nc.vector.tensor_tensor(
                            out=vp[h][:, kb * 65: kb * 65 + 64],
                            in0=ps[:, h * 64:(h + 1) * 64],
                            in1=bvb[:, h * 64:(h + 1) * 64],
                            op=ALU.add,
                        )

            # ---- attention for one pair, one query block ----
            def attention_nb(hp, nb):
                accs = [
                    psp.tile([65, 512], f32, tag="acc", bufs=2, name="acc")
                    for lh in range(2)
                ]
                for kb in range(NKT):
                    ss = psp.tile([128, 1024], f32, tag="ss", bufs=2, name="ss")
                    for lh in range(2):
                        r0 = lh * 64
                        nc.tensor.matmul(
                            ss[:, lh * 512:(lh + 1) * 512],
                            kT[hp][r0:r0 + 64, kb * 128:(kb + 1) * 128],
                            qT[hp][r0:r0 + 64, nb * 512:nb * 512 + 512],
                            start=True, stop=True,
                        )
                    ex = ep.tile([128, 1024], bf16, tag="ex", name="ex")
                    nc.scalar.activation(
                        ex, ss, AF.Exp,
                        bias=maskb[:, kb:kb + 1], scale=0.125,
                    )
                    for lh in range(2):
                        nc.tensor.matmul(
                            accs[lh],
                            vp[hp * 2 + lh][:, kb * 65: kb * 65 + 65],
                            ex[:, lh * 512:(lh + 1) * 512],
                            start=(kb == 0), stop=(kb == NKT - 1),
                        )
                # Denominators for both heads batched into one [2,512]
                # reciprocal (DVE cost scales with free size, not
                # partitions); unnormalized acc copied out to free the
                # PSUM slots early.
                # batch the two denominators at partitions 0 and 32 so a
                # single reciprocal (cost ~ free size) covers both
                uns = []
                for lh in range(2):
                    nc.vector.tensor_copy(
                        dd[32 * lh:32 * lh + 1, :], accs[lh][64:65, :])
                    un = rp.tile([64, 512], f32, tag=f"un{lh}", name="un")
                    nc.vector.tensor_copy(un, accs[lh][0:64, :])
                    uns.append(un)
                nc.vector.reciprocal(rr, dd)
                # partition_broadcast needs start partition 0: rebase row 32
                rr1 = rp.tile([1, 512], f32, tag="rr1", name="rr1")
                nc.vector.tensor_copy(rr1, rr[32:33, :])
                for lh in range(2):
                    recb = rp.tile([64, 512], f32, tag=f"recb{lh}",
                                   name="recb")
                    nc.gpsimd.partition_broadcast(
                        recb, rr[0:1, :] if lh == 0 else rr1)
                    nc.vector.tensor_tensor(
                        out=attn[hp][lh * 64:(lh + 1) * 64,
                                     nb * 512:nb * 512 + 512],
                        in0=uns[lh],
                        in1=recb,
                        op=ALU.mult,
                    )

            def outproj_nb(nb):
                for mt in range(4 * nb, 4 * nb + 4):
                    for ob in range(2):
                        po = psp.tile([128, 512], f32, tag="ps", bufs=2,
                                      name="po")
                        for p in range(2):
                            nc.tensor.matmul(
                                po,
                                attn[p][:, mt * 128:(mt + 1) * 128],
                                wout[:, p * H + ob * 512: p * H + ob * 512 + 512],
                                start=(p == 0), stop=(p == 1),
                            )
                        ot = osp.tile([128, 512], f32, tag="ot", name="ot")
                        nc.vector.tensor_copy(ot, po)
                        nc.sync.dma_start(
                            out=out_d[mt * 128:(mt + 1) * 128,
                                      ob * 512:ob * 512 + 512],
                            in_=ot,
                        )

            # ---- schedule ----
            proj_k(0)
            proj_v()
            for nb in range(NB):
                proj_q(0, nb)
            for nb in range(NB):
                attention_nb(0, nb)
            proj_k(1)
            for nb in range(NB):
                proj_q(1, nb)
            for nb in range(NB):
                attention_nb(1, nb)
                outproj_nb(nb)

    nc.compile()
    return nc


def _get_nc(nv_pad):
    key = f"nc{nv_pad}"
    if key not in _CACHE:
        _CACHE[key] = _build(nv_pad)
    return _CACHE[key]


def _prep_in_maps(x, mask, W_qkv, b_qkv, W_out):
    """Returns (in_maps, nv_pad)."""
    idxs = [np.flatnonzero(mask[b, 0, 0, :] != 0) for b in range(B)]
    nvs = [len(i) for i in idxs]
    nv_pad = max(128, ((max(nvs) + 127) // 128) * 128)
    NKT = nv_pad // 128

    xTs, xkvTs, maskbs, voness = [], [], [], []
    for b in range(B):
        xTs.append(np.ascontiguousarray(x[b].T.astype(bfloat16)))
        xkv = np.zeros((nv_pad, H), dtype=np.float32)
        xkv[: nvs[b]] = x[b][idxs[b]]
        xkvTs.append(np.ascontiguousarray(xkv.T.astype(bfloat16)))
        mb = np.zeros(nv_pad, dtype=np.float32)
        mb[nvs[b]:] = -1e9
        maskbs.append(np.ascontiguousarray(mb.reshape(NKT, 128).T))
        vo = np.zeros(nv_pad, dtype=np.float32)
        vo[: nvs[b]] = 1.0
        voness.append(np.ascontiguousarray(vo.reshape(NKT, 128).T))

    in_maps = []
    for c in range(NCORES):
        b = c // 4
        h0 = (c % 4) * HPC
        # col order q01 | q23 | k01 | k23 (128 each)
        qk_cols = np.concatenate([
            np.arange(h0 * DK, (h0 + 2) * DK),
            np.arange((h0 + 2) * DK, (h0 + 4) * DK),
            np.arange(H + h0 * DK, H + (h0 + 2) * DK),
            np.arange(H + (h0 + 2) * DK, H + (h0 + 4) * DK),
        ])
        w_qk = np.ascontiguousarray(W_qkv[:, qk_cols].astype(bfloat16))
        v_cols = np.arange(2 * H + h0 * DK, 2 * H + (h0 + 4) * DK)
        w_v = np.ascontiguousarray(W_qkv[:, v_cols].astype(bfloat16))
        w_out = np.ascontiguousarray(
            W_out[h0 * DK:(h0 + 4) * DK, :].astype(bfloat16))
        b_qk = np.ascontiguousarray(
            b_qkv[qk_cols].reshape(4, 128).T.astype(np.float32))
        b_vb = np.ascontiguousarray(np.broadcast_to(
            b_qkv[v_cols].astype(np.float32), (128, 256)))
        in_maps.append({
            "xT": xTs[b],
            "xkvT": xkvTs[b],
            "w_qk": w_qk,
            "w_v": w_v,
            "w_out": w_out,
            "maskbias": maskbs[b],
            "validones": voness[b],
            "b_qk": b_qk,
            "b_vb": b_vb,
        })
    return in_maps, nv_pad


def _combine(partials, b_out):
    out = np.empty((B, T, H), dtype=np.float32)
    for b in range(B):
        acc = partials[4 * b].astype(np.float32)
        for i in range(1, 4):
            acc = acc + partials[4 * b + i]
        out[b] = acc + b_out[None, :]
    return out


def kernel(x, mask, W_qkv, b_qkv, W_out, b_out):
    x = np.asarray(x, dtype=np.float32)
    mask = np.asarray(mask)
    W_qkv = np.asarray(W_qkv, dtype=np.float32)
    b_qkv = np.asarray(b_qkv, dtype=np.float32)
    W_out = np.asarray(W_out, dtype=np.float32)
    b_out = np.asarray(b_out, dtype=np.float32)

    in_maps, nv_pad = _prep_in_maps(x, mask, W_qkv, b_qkv, W_out)
    nc = _get_nc(nv_pad)

    from concourse.bass_utils import run_bass_kernel_spmd

    res = run_bass_kernel_spmd(nc, in_maps, list(range(NCORES)))
    partials = [res.results[c]["out_partial"] for c in range(NCORES)]
    return _combine(partials, b_out)
